# revision 10
# baseline (speedup 1.0000x reference)
"""Trainium2 Bass kernel v2 for batched GCN (2x GCNConv + circular Conv1d).

Math per graph (N=64 nodes, S=96 feats, H=512 hidden, E=512 edges):
    deg[d]   = #edges with dst=d (incl. self loop)
    A        = Dinv (M0+I)^T Dinv,  Dinv = diag(1/sqrt(deg))
    h1       = relu((A x^T) W1^T)          # aggregate-then-project
    h2       = A (h1 W2^T)
    y        = circular_conv1d(h2, conv_w).T            # [96, 512] bf16

Key structure (per core: 64 graphs = 32 pairs = 8 quads):
  - et: edge endpoints transposed once to [epos, (c, graph-endpoint)].
  - oh: fused broadcast is_equal per pair on GPSIMD -> [128, 1024] one-hots.
  - maug4: per quad [128=(gl,d), (pair,64=s)] counts+I psum via one-hot
    matmuls; deg/sqrt/dinv/row-scale batched per quad.
  - msfinS: per pair stacked [128=(gl,s), 64=(d)] = (Dinv(M0+I)^T)^T built
    with two identity matmuls (K/M offset 64 for g1).
  - gcn1: aggX^T = (A x)^T via 2 K=64 matmuls (dinv_src folded into the
    per-pair x cast), then h1^T = W1-chunksT @ aggX (4 matmuls).
  - gcn2: z2 (4 matmuls), dinv-scaled copy, agg2 = 2 K=64 matmuls.
  - conv: 3 shifted-tap matmuls per graph on dup [h2|h2]; y copied bf16
    into a resident staging tile; one batched DMA per 8 graphs.
"""

import numpy as np
import ml_dtypes

import concourse.bacc as bacc
import concourse.mybir as mybir
import concourse.tile as tile
from concourse.bass_utils import run_bass_kernel_spmd

BF16 = mybir.dt.bfloat16
FP32 = mybir.dt.float32
I32 = mybir.dt.int32
AF = mybir.ActivationFunctionType

N_CORES = 8
B, S, N, H, E = 512, 96, 64, 512, 512
G = B // N_CORES          # graphs per core
NPAIR = G // 2
NQUAD = NPAIR // 4


def build_gcn_kernel(tc, outs, ins, g=G, has_b1=False, has_b2=False):
    nc = tc.nc
    npair = g // 2
    nquad = npair // 4

    x_d = ins["x"]          # [npair, 2, 64, 96] f32 (g,n,s transposed on host)
    et_d = ins["et"]        # [128, 4*2g] bf16 (host-transposed edge table)
    w1t_d = ins["w1t"]      # [96, 512] bf16
    w2t_d = ins["w2t"]      # [128, 384] bf16 (f=(c,s))
    cwd_d = ins["cwd"]      # [128, 1536] bf16 (rows 0-63 = [i,(k,o)], dup)
    iota_d = ins["iota"]    # [128, 1024] bf16 (f%64)
    i64d_d = ins["i64d"]    # [128, 64] bf16 (I64 stacked twice)
    id128_d = ins["id128"]  # [128, 128] bf16
    y_d = outs["y"]         # [g, 96, 512] bf16

    from contextlib import ExitStack
    ctx = ExitStack()
    const = ctx.enter_context(tc.tile_pool(name="const", bufs=1))
    sb = ctx.enter_context(tc.tile_pool(name="sb", bufs=4))
    sbq = ctx.enter_context(tc.tile_pool(name="sbq", bufs=2))
    psq = ctx.enter_context(tc.tile_pool(name="psq", bufs=2, space="PSUM"))
    psmix = ctx.enter_context(tc.tile_pool(name="psmix", bufs=3,
                                           space="PSUM"))
    ps1 = ctx.enter_context(tc.tile_pool(name="ps1", bufs=1, space="PSUM"))
    psy = ctx.enter_context(tc.tile_pool(name="psy", bufs=2, space="PSUM"))

    # ---- constants (edge path first so the A-chain starts ASAP) ----
    et = const.tile([128, 4 * 2 * g], BF16)   # f = (c, gt)
    nc.sync.dma_start(out=et[:], in_=et_d[:])
    iota = const.tile([128, 1024], BF16)
    nc.sync.dma_start(out=iota[:], in_=iota_d[:])
    i64d = const.tile([128, 64], BF16)
    nc.sync.dma_start(out=i64d[:], in_=i64d_d[:])
    w1t = const.tile([96, 512], BF16)
    nc.sync.dma_start(out=w1t[:], in_=w1t_d[:])
    w2t = const.tile([128, 384], BF16)
    nc.sync.dma_start(out=w2t[:], in_=w2t_d[:])
    cwd = const.tile([128, 1536], BF16)
    nc.sync.dma_start(out=cwd[:], in_=cwd_d[:])
    ones1 = const.tile([128, 1], FP32)
    nc.gpsimd.memset(ones1[:], 1.0)
    # dummy Sqrt first so a single act-func set (sqrt incl. copy/relu) loads
    sqpin = const.tile([1, 1], FP32)
    nc.scalar.activation(out=sqpin[:], in_=iota[0:1, 0:1], func=AF.Sqrt)
    if has_b1:
        b1c = const.tile([128, 4], FP32)
        nc.sync.dma_start(out=b1c[:], in_=ins["b1c"][:])
    if has_b2:
        b2d = const.tile([128, 192], BF16)
        nc.sync.dma_start(out=b2d[:], in_=ins["b2d"][:])

    # ---- x: load f32, laid out [(gl n), (pair, s)]; cast per pair w/ dinv ----
    xt = const.tile([128, 96 * npair], FP32)
    npc = npair // 4
    for ch in range(4):
        nc.sync.dma_start(
            out=xt[:, 96 * npc * ch:96 * npc * (ch + 1)].rearrange(
                "p (pr s) -> p pr s", pr=npc),
            in_=x_d[npc * ch:npc * (ch + 1)].rearrange(
                "pr gl n s -> (gl n) pr s"))

    # ---- one-time bf16 cast of x (dinv folded into Ms instead) ----
    xtb = const.tile([128, 96 * npair], BF16)
    for ch in range(4):
        sl = slice(96 * npc * ch, 96 * npc * (ch + 1))
        nc.gpsimd.tensor_copy(out=xtb[:, sl], in_=xt[:, sl])

    # ---- output staging (bf16) ----
    ybig = const.tile([96, 512 * g], BF16)

    # ---- persistent block-diag Ms scratch (off-diag zeroed once) ----
    msbs = []
    for r in range(3):
        msb_r = const.tile([128, 128], BF16, name=f"msb{r}")
        nc.gpsimd.memset(msb_r[:], 0)
        msbs.append(msb_r)


    for q in range(nquad):
        # ===== A-chain: counts (d-part) + reversed counts (s-part) =====
        maugB = psq.tile([128, 512], FP32, tag="maugB")
        maug4 = maugB[:, 0:256]
        maugT4 = maugB[:, 256:512]
        ohs = []
        for j in range(4):
            pr = 4 * q + j
            # oh[p, (c, t, v)] = (et[p, (c, 4pr+t)] == v), fused on GPSIMD
            e_sl = et[:].rearrange("p (c gt) -> p c gt", c=4)
            e_sl = e_sl[:, :, 4 * pr:4 * pr + 4]
            e_bc = e_sl.rearrange("p c (t u) -> p c t u", u=1)
            e_bc = e_bc.to_broadcast([128, 4, 4, 64])
            erep = sb.tile([128, 1024], BF16, tag=f"erep{j}")
            if pr < 2:
                nc.gpsimd.tensor_copy(
                    out=erep[:].rearrange("p (c t v) -> p c t v",
                                          c=4, t=4),
                    in_=e_bc)
            else:
                nc.gpsimd.tensor_copy(
                    out=erep[:, 0:768].rearrange("p (c t v) -> p c t v",
                                                 c=3, t=4),
                    in_=e_bc[:, 0:3])
                nc.scalar.activation(
                    out=erep[:, 768:1024].rearrange("p (c t v) -> p c t v",
                                                    c=1, t=4),
                    in_=e_bc[:, 3:4], func=AF.Copy)
            oh = sb.tile([128, 1024], BF16, tag=f"oh{j}")
            nc.vector.tensor_tensor(
                out=oh[:], in0=erep[:], in1=iota[:],
                op=mybir.AluOpType.is_equal,
            )
            ohs.append(oh)

            # counts into maug4 (lhsT=dst, no +I: deg gets +1 via sqrt
            # bias) and counts+I into maugT4 (lhsT=src; R' needs the +I)
            for rev in range(2):
                out_sl = (maugT4 if rev else maug4)[:, 64 * j:64 * (j + 1)]
                for gl in range(2):
                    po = 64 * gl
                    dst_sl = out_sl[po:po + 64, :]  # noqa
                    tp = None if gl == 0 else (0, 64)
                    for c in range(4):
                        base = c * 256
                        a0 = oh[:, base + (2 * gl + 1) * 64:
                                base + (2 * gl + 2) * 64]
                        a1 = oh[:, base + (2 * gl) * 64:
                                base + (2 * gl + 1) * 64]
                        lhsT, rhs = (a1, a0) if rev else (a0, a1)
                        nc.tensor.matmul(dst_sl, lhsT, rhs, start=(c == 0),
                                         stop=(not rev and c == 3),
                                         tile_position=tp)
                    if rev:
                        nc.tensor.matmul(
                            dst_sl, i64d[po:po + 64, :], i64d[po:po + 64, :],
                            start=False, stop=True,
                            tile_position=None if gl == 0 else (64, 64),
                        )

        # deg -> dinv (batched over the 4 pairs)
        deg4 = sbq.tile([128, 4], FP32, tag="deg4")
        nc.vector.tensor_reduce(
            out=deg4[:].rearrange("p (q u) -> p q u", u=1),
            in_=maug4[:].rearrange("p (q s) -> p q s", s=64),
            axis=mybir.AxisListType.X, op=mybir.AluOpType.add)
        sq4 = sbq.tile([128, 4], FP32, tag="sq4")
        nc.scalar.activation(out=sq4[:], in_=deg4[:], func=AF.Sqrt,
                             bias=ones1[:, 0:1])
        dinv4 = sbq.tile([128, 4], FP32, tag="dinv4")
        nc.vector.reciprocal(out=dinv4[:], in_=sq4[:])

        for j in range(4):
            pr = 4 * q + j
            # per-pair psum: agx 0:128, z2 128:224; a2 overlaps agx 0:96
            mix = psmix.tile([128, 224], FP32, tag="mix")

            # ---- block-diag R = dinv_s * (M0+I): quadrant copies ----
            msb = msbs[pr % 3]
            nc.vector.tensor_scalar(
                out=msb[0:64, 0:64], in0=maugT4[0:64, 64 * j:64 * j + 64],
                scalar1=dinv4[0:64, j:j + 1], scalar2=None,
                op0=mybir.AluOpType.mult)
            nc.scalar.activation(
                out=msb[64:128, 64:128],
                in_=maugT4[64:128, 64 * j:64 * j + 64],
                func=AF.Copy, scale=dinv4[64:128, j:j + 1])

            # ---- aggX^T = (R^T x')^T : [96 (s), 128 (gl,d)] ----
            agx = mix[0:96, 0:128]
            nc.tensor.matmul(agx, xtb[:, 96 * pr:96 * (pr + 1)], msb[:],
                             start=True, stop=True)
            axs = sb.tile([96, 128], BF16, tag="axs")
            nc.scalar.activation(out=axs[:], in_=agx[:], func=AF.Copy)

            # ---- h1^T: [128 (h_sub), (c, gl, n)] ----
            a1t = ps1.tile([128, 512], FP32, tag="a1t")
            for c in range(4):
                nc.tensor.matmul(a1t[:, 128 * c:128 * (c + 1)],
                                 w1t[:, 128 * c:128 * (c + 1)], axs[:],
                                 start=True, stop=True)
            h1t = sb.tile([128, 512], BF16, tag="h1t")
            if has_b1:
                for c in range(4):
                    nc.scalar.activation(
                        out=h1t[:, 128 * c:128 * (c + 1)],
                        in_=a1t[:, 128 * c:128 * (c + 1)],
                        func=AF.Relu, bias=b1c[:, c:c + 1])
            else:
                nc.vector.tensor_scalar_max(h1t[:, 0:256], a1t[:, 0:256], 0.0)
                nc.scalar.activation(out=h1t[:, 256:512], in_=a1t[:, 256:512],
                                     func=AF.Relu)

            # ---- z2 = h1 W2^T: [128 (gl,n), 96 (l)] ----
            z2 = mix[:, 128:224]
            for c in range(4):
                nc.tensor.matmul(z2, h1t[:, 128 * c:128 * (c + 1)],
                                 w2t[:, 96 * c:96 * (c + 1)],
                                 start=(c == 0), stop=(c == 3))
            z2s = sb.tile([128, 96], BF16, tag="z2s")
            nc.vector.tensor_scalar(
                out=z2s[:], in0=z2, scalar1=dinv4[:, j:j + 1], scalar2=None,
                op0=mybir.AluOpType.mult)

            # ---- agg2: [128 (gl,d), 96 (l)] ----
            a2 = mix[:, 0:96]
            nc.tensor.matmul(a2, msb[:], z2s[:], start=True, stop=True)

            # ---- h2 duplicated [h2|h2] (+b2) via broadcast copy ----
            hp = sb.tile([128, 192], BF16, tag="hp")
            a2_bc = a2.rearrange("p (u f) -> p u f", u=1)
            a2_bc = a2_bc.to_broadcast([128, 2, 96])
            nc.scalar.activation(
                out=hp[:].rearrange("p (u f) -> p u f", u=2), in_=a2_bc,
                func=AF.Copy, scale=dinv4[:, j:j + 1])
            if has_b2:
                hpb = sb.tile([128, 192], BF16, tag="hpb")
                nc.vector.tensor_tensor(out=hpb[:], in0=hp[:], in1=b2d[:],
                                        op=mybir.AluOpType.add)
                hp = hpb

            # ---- conv: per graph 3 shifted-tap matmuls -> [96 (l), 512 (o)]
            for gl in range(2):
                po = 64 * gl
                gid = 2 * pr + gl
                y_ps = psy.tile([96, 512], FP32, tag="y")
                for k in range(3):
                    tap = (95, 0, 1)[k]
                    nc.tensor.matmul(
                        y_ps[:],
                        hp[po:po + 64, tap:tap + 96],
                        cwd[po:po + 64, 512 * k:512 * (k + 1)],
                        start=(k == 0), stop=(k == 2))
                ysl = ybig[:, 512 * gid:512 * (gid + 1)]
                if gl == 0:
                    nc.vector.tensor_copy(out=ysl, in_=y_ps[:])
                else:
                    nc.scalar.activation(out=ysl, in_=y_ps[:], func=AF.Copy)

        # ---- batched output DMA (last quad split for a shorter tail) ----
        g0 = 8 * q
        if q == nquad - 1:
            for h in range(2):
                nc.sync.dma_start(
                    out=y_d[g0 + 4 * h:g0 + 4 * (h + 1)].rearrange(
                        "g s o -> s g o"),
                    in_=ybig[:, 512 * (g0 + 4 * h):512 * (g0 + 4 * (h + 1))
                             ].rearrange("s (g o) -> s g o", o=512))
        else:
            nc.sync.dma_start(
                out=y_d[g0:g0 + 8].rearrange("g s o -> s g o"),
                in_=ybig[:, 512 * g0:512 * (g0 + 8)].rearrange(
                    "s (g o) -> s g o", o=512))

    ctx.close()


# ---------------- host side ----------------

def _prep_consts(W1, b1, W2, b2, conv_w):
    bf = ml_dtypes.bfloat16
    w1t = np.ascontiguousarray(W1.T).astype(bf)                    # [96, 512]
    w2t = np.ascontiguousarray(
        W2.T.reshape(4, 128, 96).transpose(1, 0, 2).reshape(128, 384)
    ).astype(bf)
    base = np.ascontiguousarray(conv_w.transpose(1, 2, 0)).reshape(64, 1536)
    cwd = np.concatenate([base, base], axis=0).astype(bf)          # [128, 1536]
    iota = np.broadcast_to((np.arange(1024) % 64).astype(bf), (128, 1024))
    iota = np.ascontiguousarray(iota)
    i64d = np.concatenate([np.eye(64), np.eye(64)], axis=0).astype(bf)
    id128 = np.eye(128).astype(bf)
    consts = dict(w1t=w1t, w2t=w2t, cwd=cwd, iota=iota, i64d=i64d,
                  id128=id128)
    has_b1 = bool(np.any(b1))
    has_b2 = bool(np.any(b2))
    if has_b1:
        consts["b1c"] = np.ascontiguousarray(
            b1.reshape(4, 128).T).astype(np.float32)
    if has_b2:
        b2d = np.ascontiguousarray(
            np.broadcast_to(np.tile(b2, 2).astype(bf), (128, 192)))
        consts["b2d"] = b2d
    return consts, has_b1, has_b2


_NC_CACHE = {}


def _get_nc(g_per_core, has_b1, has_b2):
    key = (g_per_core, has_b1, has_b2)
    if key in _NC_CACHE:
        return _NC_CACHE[key]
    nc = bacc.Bacc("TRN2", target_bir_lowering=False, debug=False)
    npair = g_per_core // 2
    ins = {
        "x": nc.dram_tensor("x", [npair, 2, 64, 96], FP32,
                            kind="ExternalInput").ap(),
        "et": nc.dram_tensor("et", [128, 4 * 2 * g_per_core], BF16,
                             kind="ExternalInput").ap(),
        "w1t": nc.dram_tensor("w1t", [96, 512], BF16,
                              kind="ExternalInput").ap(),
        "w2t": nc.dram_tensor("w2t", [128, 384], BF16,
                              kind="ExternalInput").ap(),
        "cwd": nc.dram_tensor("cwd", [128, 1536], BF16,
                              kind="ExternalInput").ap(),
        "iota": nc.dram_tensor("iota", [128, 1024], BF16,
                               kind="ExternalInput").ap(),
        "i64d": nc.dram_tensor("i64d", [128, 64], BF16,
                               kind="ExternalInput").ap(),
        "id128": nc.dram_tensor("id128", [128, 128], BF16,
                                kind="ExternalInput").ap(),
    }
    if has_b1:
        ins["b1c"] = nc.dram_tensor("b1c", [128, 4], FP32,
                                    kind="ExternalInput").ap()
    if has_b2:
        ins["b2d"] = nc.dram_tensor("b2d", [128, 192], BF16,
                                    kind="ExternalInput").ap()
    outs = {
        "y": nc.dram_tensor("y", [g_per_core, 96, 512], BF16,
                            kind="ExternalOutput").ap(),
    }
    with tile.TileContext(nc) as tc:
        build_gcn_kernel(tc, outs, ins, g_per_core, has_b1, has_b2)
    nc.compile()
    _NC_CACHE[key] = nc
    return nc


def kernel(x, edge_index, W1, b1, W2, b2, conv_w, _trace=False):
    x = np.asarray(x)
    edge_index = np.asarray(edge_index)
    consts, has_b1, has_b2 = _prep_consts(
        np.asarray(W1), np.asarray(b1), np.asarray(W2), np.asarray(b2),
        np.asarray(conv_w))
    nc = _get_nc(G, has_b1, has_b2)

    in_maps = []
    for c in range(N_CORES):
        sl = slice(c * G, (c + 1) * G)
        m = dict(consts)
        m["x"] = np.ascontiguousarray(
            x[sl].transpose(0, 2, 1).reshape(NPAIR, 2, 64, 96)
        ).astype(np.float32)
        m["et"] = np.ascontiguousarray(
            edge_index[sl].reshape(G, 2, 4, 128).transpose(3, 2, 0, 1)
            .reshape(128, 512)).astype(ml_dtypes.bfloat16)
        in_maps.append(m)

    res = run_bass_kernel_spmd(nc, in_maps, core_ids=list(range(N_CORES)),
                               trace=_trace)
    y = np.concatenate(
        [res.results[c]["y"].astype(np.float32) for c in range(N_CORES)],
        axis=0)
    if _trace:
        kernel.last_results = res
    return y


# revision 11
# speedup vs baseline: 1.0591x; 1.0591x over previous
"""Trainium2 Bass kernel v2 for batched GCN (2x GCNConv + circular Conv1d).

Math per graph (N=64 nodes, S=96 feats, H=512 hidden, E=512 edges):
    deg[d]   = #edges with dst=d (incl. self loop)
    A        = Dinv (M0+I)^T Dinv,  Dinv = diag(1/sqrt(deg))
    h1       = relu((A x^T) W1^T)          # aggregate-then-project
    h2       = A (h1 W2^T)
    y        = circular_conv1d(h2, conv_w).T            # [96, 512] bf16

Key structure (per core: 64 graphs = 32 pairs = 8 quads):
  - et: edge endpoints transposed once to [epos, (c, graph-endpoint)].
  - oh: fused broadcast is_equal per pair on GPSIMD -> [128, 1024] one-hots.
  - maug4: per quad [128=(gl,d), (pair,64=s)] counts+I psum via one-hot
    matmuls; deg/sqrt/dinv/row-scale batched per quad.
  - msfinS: per pair stacked [128=(gl,s), 64=(d)] = (Dinv(M0+I)^T)^T built
    with two identity matmuls (K/M offset 64 for g1).
  - gcn1: aggX^T = (A x)^T via 2 K=64 matmuls (dinv_src folded into the
    per-pair x cast), then h1^T = W1-chunksT @ aggX (4 matmuls).
  - gcn2: z2 (4 matmuls), dinv-scaled copy, agg2 = 2 K=64 matmuls.
  - conv: 3 shifted-tap matmuls per graph on dup [h2|h2]; y copied bf16
    into a resident staging tile; one batched DMA per 8 graphs.
"""

import numpy as np
import ml_dtypes

import concourse.bacc as bacc
import concourse.mybir as mybir
import concourse.tile as tile
from concourse.bass_utils import run_bass_kernel_spmd

BF16 = mybir.dt.bfloat16
FP32 = mybir.dt.float32
I32 = mybir.dt.int32
AF = mybir.ActivationFunctionType

N_CORES = 8
B, S, N, H, E = 512, 96, 64, 512, 512
G = B // N_CORES          # graphs per core
NPAIR = G // 2
NQUAD = NPAIR // 4


def build_gcn_kernel(tc, outs, ins, g=G, has_b1=False, has_b2=False):
    nc = tc.nc
    npair = g // 2
    nquad = npair // 4

    x_d = ins["x"]          # [npair, 2, 64, 96] f32 (g,n,s transposed on host)
    et_d = ins["et"]        # [128, 4*2g] bf16 (host-transposed edge table)
    w1t_d = ins["w1t"]      # [96, 512] bf16
    w2t_d = ins["w2t"]      # [128, 384] bf16 (f=(c,s))
    cwd_d = ins["cwd"]      # [128, 1536] bf16 (rows 0-63 = [i,(k,o)], dup)
    iota_d = ins["iota"]    # [128, 1024] bf16 (f%64)
    i64d_d = ins["i64d"]    # [128, 64] bf16 (I64 stacked twice)
    id128_d = ins["id128"]  # [128, 128] bf16
    y_d = outs["y"]         # [g, 96, 512] bf16

    from contextlib import ExitStack
    ctx = ExitStack()
    const = ctx.enter_context(tc.tile_pool(name="const", bufs=1))
    sb = ctx.enter_context(tc.tile_pool(name="sb", bufs=4))
    sbq = ctx.enter_context(tc.tile_pool(name="sbq", bufs=2))
    psq = ctx.enter_context(tc.tile_pool(name="psq", bufs=2, space="PSUM"))
    psmix = ctx.enter_context(tc.tile_pool(name="psmix", bufs=3,
                                           space="PSUM"))
    ps1 = ctx.enter_context(tc.tile_pool(name="ps1", bufs=1, space="PSUM"))
    psy = ctx.enter_context(tc.tile_pool(name="psy", bufs=2, space="PSUM"))

    # ---- constants (edge path first so the A-chain starts ASAP) ----
    et = const.tile([128, 4 * 2 * g], BF16)   # f = (c, gt)
    nc.sync.dma_start(out=et[:], in_=et_d[:])
    iota = const.tile([128, 1024], BF16)
    nc.sync.dma_start(out=iota[:], in_=iota_d[:])
    i64d = const.tile([128, 64], BF16)
    nc.sync.dma_start(out=i64d[:], in_=i64d_d[:])
    w1t = const.tile([96, 512], BF16)
    nc.sync.dma_start(out=w1t[:], in_=w1t_d[:])
    w2t = const.tile([128, 384], BF16)
    nc.sync.dma_start(out=w2t[:], in_=w2t_d[:])
    cwd = const.tile([128, 1536], BF16)
    nc.sync.dma_start(out=cwd[:], in_=cwd_d[:])
    ones1 = const.tile([128, 1], FP32)
    nc.gpsimd.memset(ones1[:], 1.0)
    # dummy Sqrt first so a single act-func set (sqrt incl. copy/relu) loads
    sqpin = const.tile([1, 1], FP32)
    nc.scalar.activation(out=sqpin[:], in_=iota[0:1, 0:1], func=AF.Sqrt)
    if has_b1:
        b1c = const.tile([128, 4], FP32)
        nc.sync.dma_start(out=b1c[:], in_=ins["b1c"][:])
    if has_b2:
        b2d = const.tile([128, 192], BF16)
        nc.sync.dma_start(out=b2d[:], in_=ins["b2d"][:])

    # ---- x: load f32, laid out [(gl n), (pair, s)]; cast per pair w/ dinv ----
    xt = const.tile([128, 96 * npair], FP32)
    npc = npair // 4
    for ch in range(4):
        nc.sync.dma_start(
            out=xt[:, 96 * npc * ch:96 * npc * (ch + 1)].rearrange(
                "p (pr s) -> p pr s", pr=npc),
            in_=x_d[npc * ch:npc * (ch + 1)].rearrange(
                "pr gl n s -> (gl n) pr s"))

    # ---- one-time bf16 cast of x (dinv folded into Ms instead) ----
    xtb = const.tile([128, 96 * npair], BF16)
    for ch in range(4):
        sl = slice(96 * npc * ch, 96 * npc * (ch + 1))
        nc.gpsimd.tensor_copy(out=xtb[:, sl], in_=xt[:, sl])

    # ---- output staging (bf16) ----
    ybig = const.tile([96, 512 * g], BF16)

    # ---- persistent block-diag Ms scratch (off-diag zeroed once) ----
    msbs = []
    for r in range(3):
        msb_r = const.tile([128, 128], BF16, name=f"msb{r}")
        nc.gpsimd.memset(msb_r[:], 0)
        msbs.append(msb_r)


    for q in range(nquad):
        # ===== A-chain: counts (d-part) + reversed counts (s-part) =====
        maugB = psq.tile([128, 512], FP32, tag="maugB")
        maug4 = maugB[:, 0:256]
        maugT4 = maugB[:, 256:512]
        ohs = []
        for j in range(4):
            pr = 4 * q + j
            # oh[p, (c, t, v)] = (et[p, (c, 4pr+t)] == v), fused on GPSIMD
            e_sl = et[:].rearrange("p (c gt) -> p c gt", c=4)
            e_sl = e_sl[:, :, 4 * pr:4 * pr + 4]
            e_bc = e_sl.rearrange("p c (t u) -> p c t u", u=1)
            e_bc = e_bc.to_broadcast([128, 4, 4, 64])
            erep = sb.tile([128, 1024], BF16, tag=f"erep{j}")
            if pr < 2:
                nc.gpsimd.tensor_copy(
                    out=erep[:].rearrange("p (c t v) -> p c t v",
                                          c=4, t=4),
                    in_=e_bc)
            else:
                nc.gpsimd.tensor_copy(
                    out=erep[:, 0:512].rearrange("p (c t v) -> p c t v",
                                                 c=2, t=4),
                    in_=e_bc[:, 0:2])
                nc.scalar.activation(
                    out=erep[:, 512:1024].rearrange("p (c t v) -> p c t v",
                                                    c=2, t=4),
                    in_=e_bc[:, 2:4], func=AF.Copy)
            oh = sb.tile([128, 1024], BF16, tag=f"oh{j}")
            nc.vector.tensor_tensor(
                out=oh[:], in0=erep[:], in1=iota[:],
                op=mybir.AluOpType.is_equal,
            )
            ohs.append(oh)

            # counts into maug4 (lhsT=dst, no +I: deg gets +1 via sqrt
            # bias) and counts+I into maugT4 (lhsT=src; R' needs the +I)
            for rev in range(2):
                out_sl = (maugT4 if rev else maug4)[:, 64 * j:64 * (j + 1)]
                for gl in range(2):
                    po = 64 * gl
                    dst_sl = out_sl[po:po + 64, :]  # noqa
                    tp = None if gl == 0 else (0, 64)
                    for c in range(4):
                        base = c * 256
                        a0 = oh[:, base + (2 * gl + 1) * 64:
                                base + (2 * gl + 2) * 64]
                        a1 = oh[:, base + (2 * gl) * 64:
                                base + (2 * gl + 1) * 64]
                        lhsT, rhs = (a1, a0) if rev else (a0, a1)
                        nc.tensor.matmul(dst_sl, lhsT, rhs, start=(c == 0),
                                         stop=(not rev and c == 3),
                                         tile_position=tp)
                    if rev:
                        nc.tensor.matmul(
                            dst_sl, i64d[po:po + 64, :], i64d[po:po + 64, :],
                            start=False, stop=True,
                            tile_position=None if gl == 0 else (64, 64),
                        )

        # deg -> dinv (batched over the 4 pairs)
        deg4 = sbq.tile([128, 4], FP32, tag="deg4")
        nc.vector.tensor_reduce(
            out=deg4[:].rearrange("p (q u) -> p q u", u=1),
            in_=maug4[:].rearrange("p (q s) -> p q s", s=64),
            axis=mybir.AxisListType.X, op=mybir.AluOpType.add)
        sq4 = sbq.tile([128, 4], FP32, tag="sq4")
        nc.scalar.activation(out=sq4[:], in_=deg4[:], func=AF.Sqrt,
                             bias=ones1[:, 0:1])
        dinv4 = sbq.tile([128, 4], FP32, tag="dinv4")
        nc.vector.reciprocal(out=dinv4[:], in_=sq4[:])

        for j in range(4):
            pr = 4 * q + j
            # per-pair psum: agx 0:128, z2 128:224; a2 overlaps agx 0:96
            mix = psmix.tile([128, 224], FP32, tag="mix")

            # ---- block-diag R = dinv_s * (M0+I): quadrant copies ----
            msb = msbs[pr % 3]
            nc.vector.tensor_scalar(
                out=msb[0:64, 0:64], in0=maugT4[0:64, 64 * j:64 * j + 64],
                scalar1=dinv4[0:64, j:j + 1], scalar2=None,
                op0=mybir.AluOpType.mult)
            nc.scalar.activation(
                out=msb[64:128, 64:128],
                in_=maugT4[64:128, 64 * j:64 * j + 64],
                func=AF.Copy, scale=dinv4[64:128, j:j + 1])

            # ---- aggX^T = (R^T x')^T : [96 (s), 128 (gl,d)] ----
            agx = mix[0:96, 0:128]
            nc.tensor.matmul(agx, xtb[:, 96 * pr:96 * (pr + 1)], msb[:],
                             start=True, stop=True)
            axs = sb.tile([96, 128], BF16, tag="axs")
            nc.scalar.activation(out=axs[:], in_=agx[:], func=AF.Copy)

            # ---- h1^T: [128 (h_sub), (c, gl, n)] ----
            a1t = ps1.tile([128, 512], FP32, tag="a1t")
            for c in range(4):
                nc.tensor.matmul(a1t[:, 128 * c:128 * (c + 1)],
                                 w1t[:, 128 * c:128 * (c + 1)], axs[:],
                                 start=True, stop=True)
            h1t = sb.tile([128, 512], BF16, tag="h1t")
            if has_b1:
                for c in range(4):
                    nc.scalar.activation(
                        out=h1t[:, 128 * c:128 * (c + 1)],
                        in_=a1t[:, 128 * c:128 * (c + 1)],
                        func=AF.Relu, bias=b1c[:, c:c + 1])
            else:
                nc.vector.tensor_scalar_max(h1t[:, 0:256], a1t[:, 0:256], 0.0)
                nc.scalar.activation(out=h1t[:, 256:512], in_=a1t[:, 256:512],
                                     func=AF.Relu)

            # ---- z2 = h1 W2^T: [128 (gl,n), 96 (l)] ----
            z2 = mix[:, 128:224]
            for c in range(4):
                nc.tensor.matmul(z2, h1t[:, 128 * c:128 * (c + 1)],
                                 w2t[:, 96 * c:96 * (c + 1)],
                                 start=(c == 0), stop=(c == 3))
            z2s = sb.tile([128, 96], BF16, tag="z2s")
            nc.vector.tensor_scalar(
                out=z2s[:], in0=z2, scalar1=dinv4[:, j:j + 1], scalar2=None,
                op0=mybir.AluOpType.mult)

            # ---- agg2: [128 (gl,d), 96 (l)] ----
            a2 = mix[:, 0:96]
            nc.tensor.matmul(a2, msb[:], z2s[:], start=True, stop=True)

            # ---- h2 duplicated [h2|h2] (+b2) via broadcast copy ----
            hp = sb.tile([128, 192], BF16, tag="hp")
            a2_bc = a2.rearrange("p (u f) -> p u f", u=1)
            a2_bc = a2_bc.to_broadcast([128, 2, 96])
            nc.scalar.activation(
                out=hp[:].rearrange("p (u f) -> p u f", u=2), in_=a2_bc,
                func=AF.Copy, scale=dinv4[:, j:j + 1])
            if has_b2:
                hpb = sb.tile([128, 192], BF16, tag="hpb")
                nc.vector.tensor_tensor(out=hpb[:], in0=hp[:], in1=b2d[:],
                                        op=mybir.AluOpType.add)
                hp = hpb

            # ---- conv: per graph 3 shifted-tap matmuls -> [96 (l), 512 (o)]
            for gl in range(2):
                po = 64 * gl
                gid = 2 * pr + gl
                y_ps = psy.tile([96, 512], FP32, tag="y")
                for k in range(3):
                    tap = (95, 0, 1)[k]
                    nc.tensor.matmul(
                        y_ps[:],
                        hp[po:po + 64, tap:tap + 96],
                        cwd[po:po + 64, 512 * k:512 * (k + 1)],
                        start=(k == 0), stop=(k == 2))
                ysl = ybig[:, 512 * gid:512 * (gid + 1)]
                if gl == 0:
                    nc.vector.tensor_copy(out=ysl, in_=y_ps[:])
                else:
                    nc.scalar.activation(out=ysl, in_=y_ps[:], func=AF.Copy)

        # ---- batched output DMA (last quad split for a shorter tail) ----
        g0 = 8 * q
        if q == nquad - 1:
            for h in range(2):
                nc.sync.dma_start(
                    out=y_d[g0 + 4 * h:g0 + 4 * (h + 1)].rearrange(
                        "g s o -> s g o"),
                    in_=ybig[:, 512 * (g0 + 4 * h):512 * (g0 + 4 * (h + 1))
                             ].rearrange("s (g o) -> s g o", o=512))
        else:
            nc.sync.dma_start(
                out=y_d[g0:g0 + 8].rearrange("g s o -> s g o"),
                in_=ybig[:, 512 * g0:512 * (g0 + 8)].rearrange(
                    "s (g o) -> s g o", o=512))

    ctx.close()


# ---------------- host side ----------------

def _prep_consts(W1, b1, W2, b2, conv_w):
    bf = ml_dtypes.bfloat16
    w1t = np.ascontiguousarray(W1.T).astype(bf)                    # [96, 512]
    w2t = np.ascontiguousarray(
        W2.T.reshape(4, 128, 96).transpose(1, 0, 2).reshape(128, 384)
    ).astype(bf)
    base = np.ascontiguousarray(conv_w.transpose(1, 2, 0)).reshape(64, 1536)
    cwd = np.concatenate([base, base], axis=0).astype(bf)          # [128, 1536]
    iota = np.broadcast_to((np.arange(1024) % 64).astype(bf), (128, 1024))
    iota = np.ascontiguousarray(iota)
    i64d = np.concatenate([np.eye(64), np.eye(64)], axis=0).astype(bf)
    id128 = np.eye(128).astype(bf)
    consts = dict(w1t=w1t, w2t=w2t, cwd=cwd, iota=iota, i64d=i64d,
                  id128=id128)
    has_b1 = bool(np.any(b1))
    has_b2 = bool(np.any(b2))
    if has_b1:
        consts["b1c"] = np.ascontiguousarray(
            b1.reshape(4, 128).T).astype(np.float32)
    if has_b2:
        b2d = np.ascontiguousarray(
            np.broadcast_to(np.tile(b2, 2).astype(bf), (128, 192)))
        consts["b2d"] = b2d
    return consts, has_b1, has_b2


_NC_CACHE = {}


def _get_nc(g_per_core, has_b1, has_b2):
    key = (g_per_core, has_b1, has_b2)
    if key in _NC_CACHE:
        return _NC_CACHE[key]
    nc = bacc.Bacc("TRN2", target_bir_lowering=False, debug=False)
    npair = g_per_core // 2
    ins = {
        "x": nc.dram_tensor("x", [npair, 2, 64, 96], FP32,
                            kind="ExternalInput").ap(),
        "et": nc.dram_tensor("et", [128, 4 * 2 * g_per_core], BF16,
                             kind="ExternalInput").ap(),
        "w1t": nc.dram_tensor("w1t", [96, 512], BF16,
                              kind="ExternalInput").ap(),
        "w2t": nc.dram_tensor("w2t", [128, 384], BF16,
                              kind="ExternalInput").ap(),
        "cwd": nc.dram_tensor("cwd", [128, 1536], BF16,
                              kind="ExternalInput").ap(),
        "iota": nc.dram_tensor("iota", [128, 1024], BF16,
                               kind="ExternalInput").ap(),
        "i64d": nc.dram_tensor("i64d", [128, 64], BF16,
                               kind="ExternalInput").ap(),
        "id128": nc.dram_tensor("id128", [128, 128], BF16,
                                kind="ExternalInput").ap(),
    }
    if has_b1:
        ins["b1c"] = nc.dram_tensor("b1c", [128, 4], FP32,
                                    kind="ExternalInput").ap()
    if has_b2:
        ins["b2d"] = nc.dram_tensor("b2d", [128, 192], BF16,
                                    kind="ExternalInput").ap()
    outs = {
        "y": nc.dram_tensor("y", [g_per_core, 96, 512], BF16,
                            kind="ExternalOutput").ap(),
    }
    with tile.TileContext(nc) as tc:
        build_gcn_kernel(tc, outs, ins, g_per_core, has_b1, has_b2)
    nc.compile()
    _NC_CACHE[key] = nc
    return nc


def kernel(x, edge_index, W1, b1, W2, b2, conv_w, _trace=False):
    x = np.asarray(x)
    edge_index = np.asarray(edge_index)
    consts, has_b1, has_b2 = _prep_consts(
        np.asarray(W1), np.asarray(b1), np.asarray(W2), np.asarray(b2),
        np.asarray(conv_w))
    nc = _get_nc(G, has_b1, has_b2)

    in_maps = []
    for c in range(N_CORES):
        sl = slice(c * G, (c + 1) * G)
        m = dict(consts)
        m["x"] = np.ascontiguousarray(
            x[sl].transpose(0, 2, 1).reshape(NPAIR, 2, 64, 96)
        ).astype(np.float32)
        m["et"] = np.ascontiguousarray(
            edge_index[sl].reshape(G, 2, 4, 128).transpose(3, 2, 0, 1)
            .reshape(128, 512)).astype(ml_dtypes.bfloat16)
        in_maps.append(m)

    res = run_bass_kernel_spmd(nc, in_maps, core_ids=list(range(N_CORES)),
                               trace=_trace)
    y = np.concatenate(
        [res.results[c]["y"].astype(np.float32) for c in range(N_CORES)],
        axis=0)
    if _trace:
        kernel.last_results = res
    return y


# revision 12
# speedup vs baseline: 1.0894x; 1.0286x over previous
"""Trainium2 Bass kernel v2 for batched GCN (2x GCNConv + circular Conv1d).

Math per graph (N=64 nodes, S=96 feats, H=512 hidden, E=512 edges):
    deg[d]   = #edges with dst=d (incl. self loop)
    A        = Dinv (M0+I)^T Dinv,  Dinv = diag(1/sqrt(deg))
    h1       = relu((A x^T) W1^T)          # aggregate-then-project
    h2       = A (h1 W2^T)
    y        = circular_conv1d(h2, conv_w).T            # [96, 512] bf16

Key structure (per core: 64 graphs = 32 pairs = 8 quads):
  - et: edge endpoints transposed once to [epos, (c, graph-endpoint)].
  - oh: fused broadcast is_equal per pair on GPSIMD -> [128, 1024] one-hots.
  - maug4: per quad [128=(gl,d), (pair,64=s)] counts+I psum via one-hot
    matmuls; deg/sqrt/dinv/row-scale batched per quad.
  - msfinS: per pair stacked [128=(gl,s), 64=(d)] = (Dinv(M0+I)^T)^T built
    with two identity matmuls (K/M offset 64 for g1).
  - gcn1: aggX^T = (A x)^T via 2 K=64 matmuls (dinv_src folded into the
    per-pair x cast), then h1^T = W1-chunksT @ aggX (4 matmuls).
  - gcn2: z2 (4 matmuls), dinv-scaled copy, agg2 = 2 K=64 matmuls.
  - conv: 3 shifted-tap matmuls per graph on dup [h2|h2]; y copied bf16
    into a resident staging tile; one batched DMA per 8 graphs.
"""

import numpy as np
import ml_dtypes

import concourse.bacc as bacc
import concourse.mybir as mybir
import concourse.tile as tile
from concourse.bass_utils import run_bass_kernel_spmd

BF16 = mybir.dt.bfloat16
FP32 = mybir.dt.float32
I32 = mybir.dt.int32
AF = mybir.ActivationFunctionType

N_CORES = 8
B, S, N, H, E = 512, 96, 64, 512, 512
G = B // N_CORES          # graphs per core
NPAIR = G // 2
NQUAD = NPAIR // 4


def build_gcn_kernel(tc, outs, ins, g=G, has_b1=False, has_b2=False):
    nc = tc.nc
    npair = g // 2
    nquad = npair // 4

    x_d = ins["x"]          # [npair, 2, 64, 96] f32 (g,n,s transposed on host)
    et_d = ins["et"]        # [128, 4*2g] bf16 (host-transposed edge table)
    w1t_d = ins["w1t"]      # [96, 512] bf16
    w2t_d = ins["w2t"]      # [128, 384] bf16 (f=(c,s))
    cwd_d = ins["cwd"]      # [128, 1536] bf16 (rows 0-63 = [i,(k,o)], dup)
    iota_d = ins["iota"]    # [128, 1024] bf16 (f%64)
    i64d_d = ins["i64d"]    # [128, 64] bf16 (I64 stacked twice)
    id128_d = ins["id128"]  # [128, 128] bf16
    y_d = outs["y"]         # [g, 96, 512] bf16

    from contextlib import ExitStack
    ctx = ExitStack()
    const = ctx.enter_context(tc.tile_pool(name="const", bufs=1))
    sb = ctx.enter_context(tc.tile_pool(name="sb", bufs=4))
    sbq = ctx.enter_context(tc.tile_pool(name="sbq", bufs=2))
    psq = ctx.enter_context(tc.tile_pool(name="psq", bufs=2, space="PSUM"))
    psmix = ctx.enter_context(tc.tile_pool(name="psmix", bufs=3,
                                           space="PSUM"))
    ps1 = ctx.enter_context(tc.tile_pool(name="ps1", bufs=1, space="PSUM"))
    psy = ctx.enter_context(tc.tile_pool(name="psy", bufs=2, space="PSUM"))

    # ---- constants (edge path first so the A-chain starts ASAP) ----
    et = const.tile([128, 4 * 2 * g], BF16)   # f = (c, gt)
    nc.sync.dma_start(out=et[:], in_=et_d[:])
    iota = const.tile([128, 1024], BF16)
    nc.sync.dma_start(out=iota[:], in_=iota_d[:])
    i64d = const.tile([128, 64], BF16)
    nc.sync.dma_start(out=i64d[:], in_=i64d_d[:])
    w1t = const.tile([96, 512], BF16)
    nc.sync.dma_start(out=w1t[:], in_=w1t_d[:])
    w2t = const.tile([128, 384], BF16)
    nc.sync.dma_start(out=w2t[:], in_=w2t_d[:])
    cwd = const.tile([128, 1536], BF16)
    nc.sync.dma_start(out=cwd[:], in_=cwd_d[:])
    ones1 = const.tile([128, 1], FP32)
    nc.gpsimd.memset(ones1[:], 1.0)
    # dummy Sqrt first so a single act-func set (sqrt incl. copy/relu) loads
    sqpin = const.tile([1, 1], FP32)
    nc.scalar.activation(out=sqpin[:], in_=iota[0:1, 0:1], func=AF.Sqrt)
    if has_b1:
        b1c = const.tile([128, 4], FP32)
        nc.sync.dma_start(out=b1c[:], in_=ins["b1c"][:])
    if has_b2:
        b2d = const.tile([128, 192], BF16)
        nc.sync.dma_start(out=b2d[:], in_=ins["b2d"][:])

    # ---- x: load f32, laid out [(gl n), (pair, s)]; cast per pair w/ dinv ----
    xt = const.tile([128, 96 * npair], FP32)
    npc = npair // 4
    for ch in range(4):
        nc.sync.dma_start(
            out=xt[:, 96 * npc * ch:96 * npc * (ch + 1)].rearrange(
                "p (pr s) -> p pr s", pr=npc),
            in_=x_d[npc * ch:npc * (ch + 1)].rearrange(
                "pr gl n s -> (gl n) pr s"))

    # ---- one-time bf16 cast of x (dinv folded into Ms instead) ----
    xtb = const.tile([128, 96 * npair], BF16)
    for ch in range(4):
        sl = slice(96 * npc * ch, 96 * npc * (ch + 1))
        eng = (nc.vector.tensor_copy, nc.gpsimd.tensor_copy)[ch % 2]
        eng(out=xtb[:, sl], in_=xt[:, sl])

    # ---- output staging (bf16) ----
    ybig = const.tile([96, 512 * g], BF16)

    # ---- persistent block-diag Ms scratch (off-diag zeroed once) ----
    msbs = []
    for r in range(3):
        msb_r = const.tile([128, 128], BF16, name=f"msb{r}")
        nc.gpsimd.memset(msb_r[:], 0)
        msbs.append(msb_r)


    for q in range(nquad):
        # ===== A-chain: counts (d-part) + reversed counts (s-part) =====
        maugB = psq.tile([128, 512], FP32, tag="maugB")
        maug4 = maugB[:, 0:256]
        maugT4 = maugB[:, 256:512]
        ohs = []
        for j in range(4):
            pr = 4 * q + j
            # oh[p, (c, t, v)] = (et[p, (c, 4pr+t)] == v), fused on GPSIMD
            e_sl = et[:].rearrange("p (c gt) -> p c gt", c=4)
            e_sl = e_sl[:, :, 4 * pr:4 * pr + 4]
            e_bc = e_sl.rearrange("p c (t u) -> p c t u", u=1)
            e_bc = e_bc.to_broadcast([128, 4, 4, 64])
            erep = sb.tile([128, 1024], BF16, tag=f"erep{j}")
            if pr < 2:
                nc.gpsimd.tensor_copy(
                    out=erep[:].rearrange("p (c t v) -> p c t v",
                                          c=4, t=4),
                    in_=e_bc)
            else:
                nc.gpsimd.tensor_copy(
                    out=erep[:, 0:512].rearrange("p (c t v) -> p c t v",
                                                 c=2, t=4),
                    in_=e_bc[:, 0:2])
                nc.scalar.activation(
                    out=erep[:, 512:1024].rearrange("p (c t v) -> p c t v",
                                                    c=2, t=4),
                    in_=e_bc[:, 2:4], func=AF.Copy)
            oh = sb.tile([128, 1024], BF16, tag=f"oh{j}")
            nc.vector.tensor_tensor(
                out=oh[:], in0=erep[:], in1=iota[:],
                op=mybir.AluOpType.is_equal,
            )
            ohs.append(oh)

            # counts into maug4 (lhsT=dst, no +I: deg gets +1 via sqrt
            # bias) and counts+I into maugT4 (lhsT=src; R' needs the +I)
            for rev in range(2):
                out_sl = (maugT4 if rev else maug4)[:, 64 * j:64 * (j + 1)]
                for gl in range(2):
                    po = 64 * gl
                    dst_sl = out_sl[po:po + 64, :]  # noqa
                    tp = None if gl == 0 else (0, 64)
                    for c in range(4):
                        base = c * 256
                        a0 = oh[:, base + (2 * gl + 1) * 64:
                                base + (2 * gl + 2) * 64]
                        a1 = oh[:, base + (2 * gl) * 64:
                                base + (2 * gl + 1) * 64]
                        lhsT, rhs = (a1, a0) if rev else (a0, a1)
                        nc.tensor.matmul(dst_sl, lhsT, rhs, start=(c == 0),
                                         stop=(not rev and c == 3),
                                         tile_position=tp)
                    if rev:
                        nc.tensor.matmul(
                            dst_sl, i64d[po:po + 64, :], i64d[po:po + 64, :],
                            start=False, stop=True,
                            tile_position=None if gl == 0 else (64, 64),
                        )

        # deg -> dinv (batched over the 4 pairs)
        deg4 = sbq.tile([128, 4], FP32, tag="deg4")
        nc.vector.tensor_reduce(
            out=deg4[:].rearrange("p (q u) -> p q u", u=1),
            in_=maug4[:].rearrange("p (q s) -> p q s", s=64),
            axis=mybir.AxisListType.X, op=mybir.AluOpType.add)
        sq4 = sbq.tile([128, 4], FP32, tag="sq4")
        nc.scalar.activation(out=sq4[:], in_=deg4[:], func=AF.Sqrt,
                             bias=ones1[:, 0:1])
        dinv4 = sbq.tile([128, 4], FP32, tag="dinv4")
        nc.vector.reciprocal(out=dinv4[:], in_=sq4[:])

        for j in range(4):
            pr = 4 * q + j
            # per-pair psum: agx 0:128, z2 128:224; a2 overlaps agx 0:96
            mix = psmix.tile([128, 224], FP32, tag="mix")

            # ---- block-diag R = dinv_s * (M0+I): quadrant copies ----
            msb = msbs[pr % 3]
            nc.vector.tensor_scalar(
                out=msb[0:64, 0:64], in0=maugT4[0:64, 64 * j:64 * j + 64],
                scalar1=dinv4[0:64, j:j + 1], scalar2=None,
                op0=mybir.AluOpType.mult)
            nc.scalar.activation(
                out=msb[64:128, 64:128],
                in_=maugT4[64:128, 64 * j:64 * j + 64],
                func=AF.Copy, scale=dinv4[64:128, j:j + 1])

            # ---- aggX^T = (R^T x')^T : [96 (s), 128 (gl,d)] ----
            agx = mix[0:96, 0:128]
            nc.tensor.matmul(agx, xtb[:, 96 * pr:96 * (pr + 1)], msb[:],
                             start=True, stop=True)
            axs = sb.tile([96, 128], BF16, tag="axs")
            nc.scalar.activation(out=axs[:], in_=agx[:], func=AF.Copy)

            # ---- h1^T: [128 (h_sub), (c, gl, n)] ----
            a1t = ps1.tile([128, 512], FP32, tag="a1t")
            for c in range(4):
                nc.tensor.matmul(a1t[:, 128 * c:128 * (c + 1)],
                                 w1t[:, 128 * c:128 * (c + 1)], axs[:],
                                 start=True, stop=True)
            h1t = sb.tile([128, 512], BF16, tag="h1t")
            if has_b1:
                for c in range(4):
                    nc.scalar.activation(
                        out=h1t[:, 128 * c:128 * (c + 1)],
                        in_=a1t[:, 128 * c:128 * (c + 1)],
                        func=AF.Relu, bias=b1c[:, c:c + 1])
            else:
                nc.vector.tensor_scalar_max(h1t[:, 0:256], a1t[:, 0:256], 0.0)
                nc.scalar.activation(out=h1t[:, 256:512], in_=a1t[:, 256:512],
                                     func=AF.Relu)

            # ---- z2 = h1 W2^T: [128 (gl,n), 96 (l)] ----
            z2 = mix[:, 128:224]
            for c in range(4):
                nc.tensor.matmul(z2, h1t[:, 128 * c:128 * (c + 1)],
                                 w2t[:, 96 * c:96 * (c + 1)],
                                 start=(c == 0), stop=(c == 3))
            z2s = sb.tile([128, 96], BF16, tag="z2s")
            nc.vector.tensor_scalar(
                out=z2s[:], in0=z2, scalar1=dinv4[:, j:j + 1], scalar2=None,
                op0=mybir.AluOpType.mult)

            # ---- agg2: [128 (gl,d), 96 (l)] ----
            a2 = mix[:, 0:96]
            nc.tensor.matmul(a2, msb[:], z2s[:], start=True, stop=True)

            # ---- h2 duplicated [h2|h2] (+b2) via broadcast copy ----
            hp = sb.tile([128, 192], BF16, tag="hp")
            a2_bc = a2.rearrange("p (u f) -> p u f", u=1)
            a2_bc = a2_bc.to_broadcast([128, 2, 96])
            nc.scalar.activation(
                out=hp[:].rearrange("p (u f) -> p u f", u=2), in_=a2_bc,
                func=AF.Copy, scale=dinv4[:, j:j + 1])
            if has_b2:
                hpb = sb.tile([128, 192], BF16, tag="hpb")
                nc.vector.tensor_tensor(out=hpb[:], in0=hp[:], in1=b2d[:],
                                        op=mybir.AluOpType.add)
                hp = hpb

            # ---- conv: per graph 3 shifted-tap matmuls -> [96 (l), 512 (o)]
            for gl in range(2):
                po = 64 * gl
                gid = 2 * pr + gl
                y_ps = psy.tile([96, 512], FP32, tag="y")
                for k in range(3):
                    tap = (95, 0, 1)[k]
                    nc.tensor.matmul(
                        y_ps[:],
                        hp[po:po + 64, tap:tap + 96],
                        cwd[po:po + 64, 512 * k:512 * (k + 1)],
                        start=(k == 0), stop=(k == 2))
                ysl = ybig[:, 512 * gid:512 * (gid + 1)]
                if gl == 0:
                    nc.vector.tensor_copy(out=ysl, in_=y_ps[:])
                else:
                    nc.scalar.activation(out=ysl, in_=y_ps[:], func=AF.Copy)

        # ---- batched output DMA (last quad split for a shorter tail) ----
        g0 = 8 * q
        if q == nquad - 1:
            for h in range(2):
                nc.sync.dma_start(
                    out=y_d[g0 + 4 * h:g0 + 4 * (h + 1)].rearrange(
                        "g s o -> s g o"),
                    in_=ybig[:, 512 * (g0 + 4 * h):512 * (g0 + 4 * (h + 1))
                             ].rearrange("s (g o) -> s g o", o=512))
        else:
            nc.sync.dma_start(
                out=y_d[g0:g0 + 8].rearrange("g s o -> s g o"),
                in_=ybig[:, 512 * g0:512 * (g0 + 8)].rearrange(
                    "s (g o) -> s g o", o=512))

    ctx.close()


# ---------------- host side ----------------

def _prep_consts(W1, b1, W2, b2, conv_w):
    bf = ml_dtypes.bfloat16
    w1t = np.ascontiguousarray(W1.T).astype(bf)                    # [96, 512]
    w2t = np.ascontiguousarray(
        W2.T.reshape(4, 128, 96).transpose(1, 0, 2).reshape(128, 384)
    ).astype(bf)
    base = np.ascontiguousarray(conv_w.transpose(1, 2, 0)).reshape(64, 1536)
    cwd = np.concatenate([base, base], axis=0).astype(bf)          # [128, 1536]
    iota = np.broadcast_to((np.arange(1024) % 64).astype(bf), (128, 1024))
    iota = np.ascontiguousarray(iota)
    i64d = np.concatenate([np.eye(64), np.eye(64)], axis=0).astype(bf)
    id128 = np.eye(128).astype(bf)
    consts = dict(w1t=w1t, w2t=w2t, cwd=cwd, iota=iota, i64d=i64d,
                  id128=id128)
    has_b1 = bool(np.any(b1))
    has_b2 = bool(np.any(b2))
    if has_b1:
        consts["b1c"] = np.ascontiguousarray(
            b1.reshape(4, 128).T).astype(np.float32)
    if has_b2:
        b2d = np.ascontiguousarray(
            np.broadcast_to(np.tile(b2, 2).astype(bf), (128, 192)))
        consts["b2d"] = b2d
    return consts, has_b1, has_b2


_NC_CACHE = {}


def _get_nc(g_per_core, has_b1, has_b2):
    key = (g_per_core, has_b1, has_b2)
    if key in _NC_CACHE:
        return _NC_CACHE[key]
    nc = bacc.Bacc("TRN2", target_bir_lowering=False, debug=False)
    npair = g_per_core // 2
    ins = {
        "x": nc.dram_tensor("x", [npair, 2, 64, 96], FP32,
                            kind="ExternalInput").ap(),
        "et": nc.dram_tensor("et", [128, 4 * 2 * g_per_core], BF16,
                             kind="ExternalInput").ap(),
        "w1t": nc.dram_tensor("w1t", [96, 512], BF16,
                              kind="ExternalInput").ap(),
        "w2t": nc.dram_tensor("w2t", [128, 384], BF16,
                              kind="ExternalInput").ap(),
        "cwd": nc.dram_tensor("cwd", [128, 1536], BF16,
                              kind="ExternalInput").ap(),
        "iota": nc.dram_tensor("iota", [128, 1024], BF16,
                               kind="ExternalInput").ap(),
        "i64d": nc.dram_tensor("i64d", [128, 64], BF16,
                               kind="ExternalInput").ap(),
        "id128": nc.dram_tensor("id128", [128, 128], BF16,
                                kind="ExternalInput").ap(),
    }
    if has_b1:
        ins["b1c"] = nc.dram_tensor("b1c", [128, 4], FP32,
                                    kind="ExternalInput").ap()
    if has_b2:
        ins["b2d"] = nc.dram_tensor("b2d", [128, 192], BF16,
                                    kind="ExternalInput").ap()
    outs = {
        "y": nc.dram_tensor("y", [g_per_core, 96, 512], BF16,
                            kind="ExternalOutput").ap(),
    }
    with tile.TileContext(nc) as tc:
        build_gcn_kernel(tc, outs, ins, g_per_core, has_b1, has_b2)
    nc.compile()
    _NC_CACHE[key] = nc
    return nc


def kernel(x, edge_index, W1, b1, W2, b2, conv_w, _trace=False):
    x = np.asarray(x)
    edge_index = np.asarray(edge_index)
    consts, has_b1, has_b2 = _prep_consts(
        np.asarray(W1), np.asarray(b1), np.asarray(W2), np.asarray(b2),
        np.asarray(conv_w))
    nc = _get_nc(G, has_b1, has_b2)

    in_maps = []
    for c in range(N_CORES):
        sl = slice(c * G, (c + 1) * G)
        m = dict(consts)
        m["x"] = np.ascontiguousarray(
            x[sl].transpose(0, 2, 1).reshape(NPAIR, 2, 64, 96)
        ).astype(np.float32)
        m["et"] = np.ascontiguousarray(
            edge_index[sl].reshape(G, 2, 4, 128).transpose(3, 2, 0, 1)
            .reshape(128, 512)).astype(ml_dtypes.bfloat16)
        in_maps.append(m)

    res = run_bass_kernel_spmd(nc, in_maps, core_ids=list(range(N_CORES)),
                               trace=_trace)
    y = np.concatenate(
        [res.results[c]["y"].astype(np.float32) for c in range(N_CORES)],
        axis=0)
    if _trace:
        kernel.last_results = res
    return y
